# revision 26
# baseline (speedup 1.0000x reference)
"""Trainium2 Bass kernel for nn_MultiHeadAttention (head-axis softmax quirk).

Strategy (8 NeuronCores), v2:
  - Data-parallel over batch (2) x query-rows (4-way) => each core owns 512 q
    rows; K/V projections sharded within each 4-core batch group and
    AllGathered per 128-key block (8 sub-gathers, ~16us each, serial on CC).
  - Emission order tuned so the CC chain starts early and hides behind the
    exp-paced attention loop: K-proj computes key-HALVES outer (kT0/kT1
    gathers fire at half-time), then Q-proj (attention can start right
    after), then V-proj per block with v_j gathers interleaved. Attention
    consumes chunks j-major; scores only need kT (arrives early) and the
    ctx 1-2 chunk lag absorbs the later v arrivals.
  - Scores row-packed: heads (2w, 2w+1) live in disjoint 64-partition halves
    (dk=64), so their K=64 matmuls run CONCURRENTLY as PE row-tiles
    (0,0)/(64,0), halving score PE time.
  - softmax over HEADS: exp per 2-head wave (ScalarE = pacer, ~8.4us/chunk);
    denominator D = sum over heads via batched identity matmuls (single
    LDWEIGHTS) into one PSUM bank; reciprocal on DVE; normalize split
    DVE (12 heads, one op) / GpSimd (4 heads).
  - ctx: head-pairs 0-1 accumulate in 2 resident PSUM banks across the whole
    k-loop (start/stop); pairs 2-7 evacuate per chunk via DVE. ctx for chunk
    c is emitted mid-way through chunk c+1's score waves (fills PE).
  - PSUM budget: score waves 2x2 banks + D 1 + resident ctx 2 + cpsx 1 = 8.
  - Output projection + residual + LayerNorm fused on-chip at the tail.
"""

import numpy as np
import ml_dtypes

D = 1024
H = 16
DK = 64
P = 128
M = 512  # q rows per core
SK = 2048  # k length per batch
G = 4  # cores per batch group
KC = SK // P
EC = D // P
HP = H // 2
LN_EPS = 1e-5

_CACHE = {}


def _build():
    from contextlib import ExitStack

    import concourse.bass as bass
    import concourse.mybir as mybir
    import concourse.tile as tile
    from concourse import bacc
    from concourse.masks import make_identity

    f32 = mybir.dt.float32
    bf16 = mybir.dt.bfloat16
    AF = mybir.ActivationFunctionType
    OP = mybir.AluOpType

    nc = bacc.Bacc("TRN2", target_bir_lowering=False, debug=False, num_devices=8)

    # weights/inputs are pre-laid-out host-side as [P, EC, *] (partition-major)
    # so every DMA is 128 fat contiguous descriptors
    xqT = nc.dram_tensor("xqT", [P, EC * M], bf16, kind="ExternalInput").ap()
    xkT = nc.dram_tensor("xkT", [P, EC * M], bf16, kind="ExternalInput").ap()
    xvT = nc.dram_tensor("xvT", [P, EC * M], bf16, kind="ExternalInput").ap()
    xres = nc.dram_tensor("xres", [M, D], f32, kind="ExternalInput").ap()
    wq = nc.dram_tensor("wq", [P, EC * D], bf16, kind="ExternalInput").ap()
    wk = nc.dram_tensor("wk", [P, EC * D], bf16, kind="ExternalInput").ap()
    wv = nc.dram_tensor("wv", [P, EC * D], bf16, kind="ExternalInput").ap()
    wo = nc.dram_tensor("wo", [P, EC * D], bf16, kind="ExternalInput").ap()
    gam = nc.dram_tensor("gam", [P, D], f32, kind="ExternalInput").ap()
    bet = nc.dram_tensor("bet", [P, D], f32, kind="ExternalInput").ap()
    out = nc.dram_tensor("out", [M, D], f32, kind="ExternalOutput").ap()

    with tile.TileContext(nc) as tc:
        with ExitStack() as ctx:
            const = ctx.enter_context(tc.tile_pool(name="const", bufs=1))
            wpool = ctx.enter_context(tc.tile_pool(name="w", bufs=3))
            xtp = ctx.enter_context(tc.tile_pool(name="xt", bufs=3))
            kvstr = ctx.enter_context(tc.tile_pool(name="kvstr", bufs=3))
            expp = ctx.enter_context(tc.tile_pool(name="expp", bufs=3))
            pevac = ctx.enter_context(tc.tile_pool(name="pevac", bufs=2))
            smal = ctx.enter_context(tc.tile_pool(name="smal", bufs=3))
            resp = ctx.enter_context(tc.tile_pool(name="resp", bufs=2))
            lnp = ctx.enter_context(tc.tile_pool(name="lnp", bufs=2))
            psum = ctx.enter_context(tc.tile_pool(name="psum", bufs=2, space="PSUM"))
            dram = ctx.enter_context(tc.tile_pool(name="dram", bufs=1, space="DRAM"))

            # persistent tiles
            id_bf = const.tile([P, P], bf16)
            make_identity(nc, id_bf[:])
            ctx_sb = const.tile([P, HP, M], f32)
            qT_sb = const.tile([P, EC, M], bf16)
            gam_sb = const.tile([P, D], f32)
            nc.sync.dma_start(gam_sb[:], gam[:])
            bet_sb = const.tile([P, D], f32)
            nc.sync.dma_start(bet_sb[:], bet[:])

            # per-key-block collective buffers; kT is partition-major
            # [p, o, k] so the per-chunk kt DMA reads 2KB/partition
            kT_in = [
                dram.tile([P, EC * P], bf16, name=f"kT_in{j}") for j in range(G)
            ]
            v_in = [dram.tile([P, D], bf16, name=f"v_in{j}") for j in range(G)]
            kT_out = [
                dram.tile([G * P, EC * P], bf16, name=f"kT_out{j}")
                for j in range(G)
            ]
            v_out = [
                dram.tile([G * P, D], bf16, name=f"v_out{j}") for j in range(G)
            ]
            rg = [[0, 1, 2, 3], [4, 5, 6, 7]]

            def gather(j, which):
                src = kT_in[j] if which == "k" else v_in[j]
                dst = kT_out[j] if which == "k" else v_out[j]
                nc.gpsimd.collective_compute(
                    "AllGather",
                    OP.bypass,
                    replica_groups=rg,
                    ins=[src.opt()],
                    outs=[dst.opt()],
                )

            # DMA all weights/inputs up front (no deps -> issue early)
            wk_sb = wpool.tile([P, EC, D], bf16, tag="w")
            nc.sync.dma_start(wk_sb[:], wk.rearrange("p (o e) -> p o e", o=EC))
            xk_sb = xtp.tile([P, EC, M], bf16, tag="xt")
            nc.sync.dma_start(xk_sb[:], xkT.rearrange("p (o q) -> p o q", o=EC))

            kin_views = [t.rearrange("p (o k) -> p o k", k=P) for t in kT_in]
            KH = M // 2  # 256 keys per half

            def kproj_half(kh):
                # K^T projection for key blocks {2kh, 2kh+1}
                ks = slice(kh * KH, (kh + 1) * KH)
                for w0 in range(0, EC, 2):
                    ps = psum.tile([P, 2, KH], f32, tag="sc")
                    for wj in range(2):
                        for dc in range(EC):
                            nc.tensor.matmul(
                                ps[:, wj, :],
                                lhsT=wk_sb[:, dc, (w0 + wj) * P : (w0 + wj + 1) * P],
                                rhs=xk_sb[:, dc, ks],
                                start=(dc == 0),
                                stop=(dc == EC - 1),
                            )
                    ev = pevac.tile([P, 2, KH], bf16, tag="pe")
                    nc.vector.tensor_copy(ev[:], ps[:])
                    for jj in range(2):
                        j = 2 * kh + jj
                        nc.sync.dma_start(
                            kin_views[j][:, w0 : w0 + 2, :],
                            ev[:, :, jj * P : (jj + 1) * P],
                        )

            def vproj_block(j, wv_sb, xv_sb):
                # V projection for key block j: v_in[j] = [128 keys, 1024 e]
                ps = psum.tile([P, 2, M], f32, tag="sc")
                for eh in range(2):
                    for dc in range(EC):
                        nc.tensor.matmul(
                            ps[:, eh, :],
                            lhsT=xv_sb[:, dc, j * P : (j + 1) * P],
                            rhs=wv_sb[:, dc, eh * M : (eh + 1) * M],
                            start=(dc == 0),
                            stop=(dc == EC - 1),
                        )
                ev = pevac.tile([P, 2, M], bf16, tag="pe")
                nc.vector.tensor_copy(ev[:], ps[:])
                nc.sync.dma_start(
                    v_in[j][:].rearrange("p (h m) -> p h m", h=2), ev[:]
                )

            # ---- Projections + gathers, ordered so the serial CC chain
            # (kT0, v0, kT1, v1, kT2, v2, kT3, v3) starts ASAP and each
            # piece lands before its consumer chunk group ----
            wv_sb = wpool.tile([P, EC, D], bf16, tag="w")
            nc.sync.dma_start(wv_sb[:], wv.rearrange("p (o e) -> p o e", o=EC))
            xv_sb = xtp.tile([P, EC, M], bf16, tag="xt")
            nc.sync.dma_start(xv_sb[:], xvT.rearrange("p (o q) -> p o q", o=EC))

            kproj_half(0)
            gather(0, "k")
            vproj_block(0, wv_sb, xv_sb)
            gather(0, "v")
            gather(1, "k")  # input ready with half 0

            # Q^T projection (local) -- before K-half2/V1-3 so attention
            # starts as early as possible
            wq_sb = wpool.tile([P, EC, D], bf16, tag="w")
            nc.sync.dma_start(wq_sb[:], wq.rearrange("p (o e) -> p o e", o=EC))
            xq_sb = xtp.tile([P, EC, M], bf16, tag="xt")
            nc.sync.dma_start(xq_sb[:], xqT.rearrange("p (o q) -> p o q", o=EC))
            for w0 in range(0, EC, 2):
                ps = psum.tile([P, 2, M], f32, tag="sc")
                for wj in range(2):
                    for dc in range(EC):
                        nc.tensor.matmul(
                            ps[:, wj, :],
                            lhsT=wq_sb[:, dc, (w0 + wj) * P : (w0 + wj + 1) * P],
                            rhs=xq_sb[:, dc, :],
                            start=(dc == 0),
                            stop=(dc == EC - 1),
                        )
                nc.vector.tensor_copy(qT_sb[:, w0 : w0 + 2, :], ps[:])

            # remaining projections + rest of the CC chain
            kproj_half(1)
            vproj_block(1, wv_sb, xv_sb)
            gather(1, "v")
            gather(2, "k")
            vproj_block(2, wv_sb, xv_sb)
            gather(2, "v")
            gather(3, "k")
            vproj_block(3, wv_sb, xv_sb)
            gather(3, "v")

            # (load W_O early; DMA overlaps the attention loop)
            wo_sb = wpool.tile([P, EC, D], bf16, tag="w")
            nc.sync.dma_start(wo_sb[:], wo.rearrange("p (o e) -> p o e", o=EC))

            # ---- Phase D: attention, chunks j-major (gather completion
            # order) ----
            kT_views = [
                t.rearrange("(g p) (o k) -> g p o k", g=G, k=P) for t in kT_out
            ]
            v_views = [t.rearrange("(g p) e -> g p e", p=P) for t in v_out]

            sched = [(g, j) for j in range(G) for g in range(G)]
            NCHUNK = len(sched)

            # resident ctx head-pairs (accumulate in PSUM across all chunks)
            N_RES = 1
            cres = [
                psum.tile([P, M], f32, tag=f"cres{hp}", bufs=1, name=f"cres{hp}")
                for hp in range(N_RES)
            ]

            def emit_ctx_pair(vt, et, ci, hp):
                # one head-pair's context matmuls (col-packed, concurrent)
                if hp < N_RES:
                    cps = cres[hp]
                    st, sp = (ci == 0), (ci == NCHUNK - 1)
                else:
                    cps = psum.tile([P, M], f32, tag="cpsx", bufs=2, name="cpsx")
                    st, sp = True, True
                nc.tensor.matmul(
                    cps[0:DK, :],
                    lhsT=vt[:, (2 * hp) * DK : (2 * hp + 1) * DK],
                    rhs=et[:, 2 * hp, :],
                    start=st,
                    stop=sp,
                    tile_position=(0, 0),
                    skip_group_check=True,
                )
                nc.tensor.matmul(
                    cps[DK:P, :],
                    lhsT=vt[:, (2 * hp + 1) * DK : (2 * hp + 2) * DK],
                    rhs=et[:, 2 * hp + 1, :],
                    start=st,
                    stop=sp,
                    tile_position=(0, 64),
                    skip_group_check=True,
                )
                if hp >= N_RES:
                    if ci == 0:
                        nc.vector.tensor_copy(ctx_sb[:, hp, :], cps[:])
                    else:
                        nc.vector.tensor_tensor(
                            ctx_sb[:, hp, :], ctx_sb[:, hp, :], cps[:], OP.add
                        )

            def finalize_a(prev):
                # reciprocal of the previous chunk's denominator (DVE)
                _, _, _, dps, rf, _ = prev
                nc.vector.reciprocal_approx_fast(rf[:], dps[:])

            def finalize_b(prev):
                # cast recip to bf16 (ScalarE) + normalize: DVE heads 0-11
                # (one op), GpSimd heads 12-15
                _, et, _, _, rf, rb = prev
                nc.scalar.copy(rb[:], rf[:])
                nc.vector.tensor_tensor(
                    et[:, :12, :],
                    et[:, :12, :],
                    rb[:, None, :].to_broadcast((P, 12, M)),
                    OP.mult,
                )
                nc.gpsimd.tensor_tensor(
                    et[:, 12:, :],
                    et[:, 12:, :],
                    rb[:, None, :].to_broadcast((P, H - 12, M)),
                    OP.mult,
                )

            prev = None
            for ci, (g, j) in enumerate(sched):
                kt = kvstr.tile([P, EC, P], bf16, tag="kt")
                nc.sync.dma_start(kt[:], kT_views[j][g])
                vt = kvstr.tile([P, D], bf16, tag="vt")
                nc.sync.dma_start(vt[:], v_views[j][g])

                et = expp.tile([P, H, M], bf16, tag="exp")
                dps = psum.tile([P, M], f32, tag="dps", bufs=1)
                # score waves: 2 heads row-packed (PE row-tiles (0,0)/(64,0)
                # run concurrently); exp on ScalarE per wave; D identity
                # matmuls for wave w-1 slot in behind wave w's scores. The
                # PREVIOUS chunk's finalize (recip/cast/normalize) rides in
                # waves 0-1 and its ctx pairs fill waves 4-7, so the exp
                # train never stalls at a chunk boundary.
                for w in range(H // 2):
                    ps = psum.tile([P, 2, M], f32, tag="sc")
                    nc.tensor.matmul(
                        ps[:, 0, :],
                        lhsT=kt[0:DK, w, :],
                        rhs=qT_sb[0:DK, w, :],
                        start=True,
                        stop=True,
                    )
                    nc.tensor.matmul(
                        ps[:, 1, :],
                        lhsT=kt[DK:P, w, :],
                        rhs=qT_sb[DK:P, w, :],
                        start=True,
                        stop=True,
                    )
                    nc.scalar.activation(
                        et[:, 2 * w : 2 * w + 2, :], ps[:], AF.Exp, scale=0.125
                    )
                    if w == 0 and prev is not None:
                        finalize_a(prev)
                    if w == 1 and prev is not None:
                        finalize_b(prev)
                    if w >= 1:
                        for hh in (2 * (w - 1), 2 * w - 1):
                            nc.tensor.matmul(
                                dps[:],
                                lhsT=id_bf[:],
                                rhs=et[:, hh, :],
                                start=(hh == 0),
                                stop=False,
                                skip_group_check=True,
                            )
                    if w >= 4 and prev is not None:
                        emit_ctx_pair(prev[0], prev[1], prev[2], 2 * (w - 4))
                        emit_ctx_pair(prev[0], prev[1], prev[2], 2 * (w - 4) + 1)

                for hh in (H - 2, H - 1):
                    nc.tensor.matmul(
                        dps[:],
                        lhsT=id_bf[:],
                        rhs=et[:, hh, :],
                        start=False,
                        stop=(hh == H - 1),
                        skip_group_check=True,
                    )
                rf = smal.tile([P, M], f32, tag="rf", bufs=2)
                rb = smal.tile([P, M], bf16, tag="rb", bufs=2)
                prev = (vt, et, ci, dps, rf, rb)

            finalize_a(prev)
            finalize_b(prev)
            for hp in range(HP):
                emit_ctx_pair(prev[0], prev[1], prev[2], hp)
            # evacuate resident pairs
            for hp in range(N_RES):
                nc.vector.tensor_copy(ctx_sb[:, hp, :], cres[hp][:])

            # ---- Phase E: output projection + residual + LayerNorm ----
            ctx_bf = const.tile([P, HP, M], bf16)
            nc.vector.tensor_copy(ctx_bf[:], ctx_sb[:])
            res_view = xres.rearrange("(o p) e -> o p e", p=P)
            out_view = out.rearrange("(o p) e -> o p e", p=P)
            for qc in range(M // P):
                rest = resp.tile([P, D], f32, tag="res")
                nc.sync.dma_start(rest[:], res_view[qc])
                xsb = lnp.tile([P, D], f32, tag="x")
                ps = psum.tile([P, 2, M], f32, tag="sc")
                for eh in range(2):
                    for vc in range(EC):
                        nc.tensor.matmul(
                            ps[:, eh, :],
                            lhsT=ctx_bf[:, vc, qc * P : (qc + 1) * P],
                            rhs=wo_sb[:, vc, eh * M : (eh + 1) * M],
                            start=(vc == 0),
                            stop=(vc == EC - 1),
                        )
                for eh in range(2):
                    nc.vector.tensor_tensor(
                        xsb[:, eh * M : (eh + 1) * M],
                        ps[:, eh, :],
                        rest[:, eh * M : (eh + 1) * M],
                        OP.add,
                    )

                # LN stats on ScalarE: copy-with-accum -> sum(x); center via
                # per-partition bias; square-with-accum -> sum((x-mu)^2).
                # The activation data outputs are throwaways dumped into
                # tiles that are dead (or about to be overwritten).
                xc = lnp.tile([P, D], f32, tag="xc", bufs=1)
                mu_r = smal.tile([P, 1], f32, tag="mu")
                nc.scalar.activation(xc[:], xsb[:], AF.Identity, accum_out=mu_r[:])
                mu_neg = smal.tile([P, 1], f32, tag="mu2")
                nc.vector.tensor_scalar_mul(mu_neg[:], mu_r[:], -1.0 / D)
                nc.scalar.activation(xc[:], xsb[:], AF.Identity, bias=mu_neg[:])
                var_r = smal.tile([P, 1], f32, tag="var")
                nc.scalar.activation(xsb[:], xc[:], AF.Square, accum_out=var_r[:])
                veps = smal.tile([P, 1], f32, tag="veps")
                nc.vector.tensor_scalar(
                    veps[:], var_r[:], 1.0 / D, LN_EPS, OP.mult, OP.add
                )
                iv2 = smal.tile([P, 1], f32, tag="iv2")
                nc.vector.reciprocal_approx_fast(iv2[:], veps[:])
                inv = smal.tile([P, 1], f32, tag="inv")
                nc.scalar.activation(inv[:], iv2[:], AF.Sqrt)
                nc.vector.scalar_tensor_tensor(
                    xc[:], xc[:], inv[:], gam_sb[:], OP.mult, OP.mult
                )
                nc.vector.tensor_tensor(xsb[:], xc[:], bet_sb[:], OP.add)
                nc.sync.dma_start(out_view[qc], xsb[:])

    nc.compile()
    return nc


def _get_nc():
    if "nc" not in _CACHE:
        _CACHE["nc"] = _build()
    return _CACHE["nc"]


def _pmaj(a2d):
    """[D_out, N] -> partition-major [P, EC*N] so device DMA reads are
    128 fat contiguous rows: row p holds [o, n] for o = D_out chunk."""
    d, n = a2d.shape
    return np.ascontiguousarray(
        a2d.reshape(d // P, P, n).transpose(1, 0, 2).reshape(P, (d // P) * n)
    )


def _in_maps(input_Q, input_K, input_V, W_Q, W_K, W_V, W_O, ln_gamma, ln_beta):
    bf = ml_dtypes.bfloat16
    f32 = np.float32
    Q_ = np.asarray(input_Q, dtype=f32)
    K_ = np.asarray(input_K, dtype=f32)
    V_ = np.asarray(input_V, dtype=f32)
    wq_b = _pmaj(np.asarray(W_Q, dtype=f32).astype(bf))
    wk_b = _pmaj(np.asarray(W_K, dtype=f32).astype(bf))
    wv_b = _pmaj(np.asarray(W_V, dtype=f32).astype(bf))
    wo_b = _pmaj(np.asarray(W_O, dtype=f32).astype(bf))
    gam_b = np.ascontiguousarray(
        np.broadcast_to(np.asarray(ln_gamma, dtype=f32), (P, D))
    )
    bet_b = np.ascontiguousarray(
        np.broadcast_to(np.asarray(ln_beta, dtype=f32), (P, D))
    )
    maps = []
    for c in range(8):
        b, r = divmod(c, G)
        sl = slice(r * M, (r + 1) * M)
        maps.append(
            {
                "xqT": _pmaj(np.ascontiguousarray(Q_[b, sl].T).astype(bf)),
                "xkT": _pmaj(np.ascontiguousarray(K_[b, sl].T).astype(bf)),
                "xvT": _pmaj(np.ascontiguousarray(V_[b, sl].T).astype(bf)),
                "xres": np.ascontiguousarray(Q_[b, sl]),
                "wq": wq_b,
                "wk": wk_b,
                "wv": wv_b,
                "wo": wo_b,
                "gam": gam_b,
                "bet": bet_b,
            }
        )
    return maps


def _assemble(results):
    B = 2
    out = np.empty((B, SK, D), np.float32)
    for c in range(8):
        b, r = divmod(c, G)
        out[b, r * M : (r + 1) * M] = results[c]["out"]
    return out


def run_traced(trace=False, **inputs):
    """Run on HW; returns (output, BassKernelResults)."""
    from concourse.bass_utils import run_bass_kernel_spmd

    nc = _get_nc()
    maps = _in_maps(**inputs)
    res = run_bass_kernel_spmd(nc, maps, list(range(8)), trace=trace)
    return _assemble(res.results), res


def kernel(**inputs) -> np.ndarray:
    out, _ = run_traced(trace=False, **inputs)
    return out


# revision 27
# speedup vs baseline: 1.2115x; 1.2115x over previous
"""Trainium2 Bass kernel for nn_MultiHeadAttention (head-axis softmax quirk).

Strategy (8 NeuronCores), v2:
  - Data-parallel over batch (2) x query-rows (4-way) => each core owns 512 q
    rows; K/V projections sharded within each 4-core batch group and
    AllGathered per 128-key block (8 sub-gathers, ~16us each, serial on CC).
  - Emission order tuned so the CC chain starts early and hides behind the
    exp-paced attention loop: K-proj computes key-HALVES outer (kT0/kT1
    gathers fire at half-time), then Q-proj (attention can start right
    after), then V-proj per block with v_j gathers interleaved. Attention
    consumes chunks j-major; scores only need kT (arrives early) and the
    ctx 1-2 chunk lag absorbs the later v arrivals.
  - Scores row-packed: heads (2w, 2w+1) live in disjoint 64-partition halves
    (dk=64), so their K=64 matmuls run CONCURRENTLY as PE row-tiles
    (0,0)/(64,0), halving score PE time.
  - softmax over HEADS: exp per 2-head wave (ScalarE = pacer, ~8.4us/chunk);
    denominator D = sum over heads via batched identity matmuls (single
    LDWEIGHTS) into one PSUM bank; reciprocal on DVE; normalize split
    DVE (12 heads, one op) / GpSimd (4 heads).
  - ctx: head-pairs 0-1 accumulate in 2 resident PSUM banks across the whole
    k-loop (start/stop); pairs 2-7 evacuate per chunk via DVE. ctx for chunk
    c is emitted mid-way through chunk c+1's score waves (fills PE).
  - PSUM budget: score waves 2x2 banks + D 1 + resident ctx 2 + cpsx 1 = 8.
  - Output projection + residual + LayerNorm fused on-chip at the tail.
"""

import numpy as np
import ml_dtypes

D = 1024
H = 16
DK = 64
P = 128
M = 512  # q rows per core
SK = 2048  # k length per batch
G = 4  # cores per batch group
KC = SK // P
EC = D // P
HP = H // 2
LN_EPS = 1e-5

_CACHE = {}


def _build():
    from contextlib import ExitStack

    import concourse.bass as bass
    import concourse.mybir as mybir
    import concourse.tile as tile
    from concourse import bacc
    from concourse.masks import make_identity

    f32 = mybir.dt.float32
    bf16 = mybir.dt.bfloat16
    AF = mybir.ActivationFunctionType
    OP = mybir.AluOpType

    nc = bacc.Bacc("TRN2", target_bir_lowering=False, debug=False, num_devices=8)

    # weights/inputs are pre-laid-out host-side as [P, EC, *] (partition-major)
    # so every DMA is 128 fat contiguous descriptors
    xqT = nc.dram_tensor("xqT", [P, EC * M], bf16, kind="ExternalInput").ap()
    xkT = nc.dram_tensor("xkT", [P, EC * M], bf16, kind="ExternalInput").ap()
    xvT = nc.dram_tensor("xvT", [P, EC * M], bf16, kind="ExternalInput").ap()
    xres = nc.dram_tensor("xres", [M, D], f32, kind="ExternalInput").ap()
    wq = nc.dram_tensor("wq", [P, EC * D], bf16, kind="ExternalInput").ap()
    wk = nc.dram_tensor("wk", [P, EC * D], bf16, kind="ExternalInput").ap()
    wv = nc.dram_tensor("wv", [P, EC * D], bf16, kind="ExternalInput").ap()
    wo = nc.dram_tensor("wo", [P, EC * D], bf16, kind="ExternalInput").ap()
    gam = nc.dram_tensor("gam", [P, D], f32, kind="ExternalInput").ap()
    bet = nc.dram_tensor("bet", [P, D], f32, kind="ExternalInput").ap()
    out = nc.dram_tensor("out", [M, D], f32, kind="ExternalOutput").ap()

    with tile.TileContext(nc) as tc:
        with ExitStack() as ctx:
            const = ctx.enter_context(tc.tile_pool(name="const", bufs=1))
            wpool = ctx.enter_context(tc.tile_pool(name="w", bufs=3))
            xtp = ctx.enter_context(tc.tile_pool(name="xt", bufs=3))
            kvstr = ctx.enter_context(tc.tile_pool(name="kvstr", bufs=3))
            expp = ctx.enter_context(tc.tile_pool(name="expp", bufs=3))
            pevac = ctx.enter_context(tc.tile_pool(name="pevac", bufs=2))
            smal = ctx.enter_context(tc.tile_pool(name="smal", bufs=3))
            resp = ctx.enter_context(tc.tile_pool(name="resp", bufs=2))
            lnp = ctx.enter_context(tc.tile_pool(name="lnp", bufs=2))
            psum = ctx.enter_context(tc.tile_pool(name="psum", bufs=2, space="PSUM"))
            dram = ctx.enter_context(tc.tile_pool(name="dram", bufs=1, space="DRAM"))

            # persistent tiles
            id_bf = const.tile([P, P], bf16)
            make_identity(nc, id_bf[:])
            ctx_sb = const.tile([P, HP, M], f32)
            qT_sb = const.tile([P, EC, M], bf16)
            gam_sb = const.tile([P, D], f32)
            nc.sync.dma_start(gam_sb[:], gam[:])
            bet_sb = const.tile([P, D], f32)
            nc.sync.dma_start(bet_sb[:], bet[:])

            # per-key-block collective buffers; kT is partition-major
            # [p, o, k] so the per-chunk kt DMA reads 2KB/partition
            kT_in = [
                dram.tile([P, EC * P], bf16, name=f"kT_in{j}") for j in range(G)
            ]
            v_in = [dram.tile([P, D], bf16, name=f"v_in{j}") for j in range(G)]
            kT_out = [
                dram.tile([G * P, EC * P], bf16, name=f"kT_out{j}")
                for j in range(G)
            ]
            v_out = [
                dram.tile([G * P, D], bf16, name=f"v_out{j}") for j in range(G)
            ]
            rg = [[0, 1, 2, 3], [4, 5, 6, 7]]

            def gather(j, which):
                src = kT_in[j] if which == "k" else v_in[j]
                dst = kT_out[j] if which == "k" else v_out[j]
                nc.gpsimd.collective_compute(
                    "AllGather",
                    OP.bypass,
                    replica_groups=rg,
                    ins=[src.opt()],
                    outs=[dst.opt()],
                )

            # DMA all weights/inputs up front (no deps -> issue early)
            wk_sb = wpool.tile([P, EC, D], bf16, tag="w")
            nc.sync.dma_start(wk_sb[:], wk.rearrange("p (o e) -> p o e", o=EC))
            xk_sb = xtp.tile([P, EC, M], bf16, tag="xt")
            nc.sync.dma_start(xk_sb[:], xkT.rearrange("p (o q) -> p o q", o=EC))

            kin_views = [t.rearrange("p (o k) -> p o k", k=P) for t in kT_in]

            # ---- Phase A: K^T projection (waves of 2 e-chunks) ----
            for w0 in range(0, EC, 2):
                ps = psum.tile([P, 2, M], f32, tag="sc")
                for wj in range(2):
                    for dc in range(EC):
                        nc.tensor.matmul(
                            ps[:, wj, :],
                            lhsT=wk_sb[:, dc, (w0 + wj) * P : (w0 + wj + 1) * P],
                            rhs=xk_sb[:, dc, :],
                            start=(dc == 0),
                            stop=(dc == EC - 1),
                        )
                ev = pevac.tile([P, 2, M], bf16, tag="pe")
                nc.vector.tensor_copy(ev[:], ps[:])
                for j in range(G):
                    nc.sync.dma_start(
                        kin_views[j][:, w0 : w0 + 2, :],
                        ev[:, :, j * P : (j + 1) * P],
                    )

            # ---- Phase B: V projection (key-block-major) + interleaved
            # kT_j/v_j sub-gathers (CC chain: k0,v0,k1,v1,...) ----
            wv_sb = wpool.tile([P, EC, D], bf16, tag="w")
            nc.sync.dma_start(wv_sb[:], wv.rearrange("p (o e) -> p o e", o=EC))
            xv_sb = xtp.tile([P, EC, M], bf16, tag="xt")
            nc.sync.dma_start(xv_sb[:], xvT.rearrange("p (o q) -> p o q", o=EC))

            for j in range(G):
                ps = psum.tile([P, 2, M], f32, tag="sc")
                for eh in range(2):
                    for dc in range(EC):
                        nc.tensor.matmul(
                            ps[:, eh, :],
                            lhsT=xv_sb[:, dc, j * P : (j + 1) * P],
                            rhs=wv_sb[:, dc, eh * M : (eh + 1) * M],
                            start=(dc == 0),
                            stop=(dc == EC - 1),
                        )
                ev = pevac.tile([P, 2, M], bf16, tag="pe")
                nc.vector.tensor_copy(ev[:], ps[:])
                nc.sync.dma_start(
                    v_in[j][:].rearrange("p (h m) -> p h m", h=2), ev[:]
                )
                gather(j, "k")
                gather(j, "v")

            # ---- Phase C: Q^T projection (local) ----
            wq_sb = wpool.tile([P, EC, D], bf16, tag="w")
            nc.sync.dma_start(wq_sb[:], wq.rearrange("p (o e) -> p o e", o=EC))
            xq_sb = xtp.tile([P, EC, M], bf16, tag="xt")
            nc.sync.dma_start(xq_sb[:], xqT.rearrange("p (o q) -> p o q", o=EC))
            for w0 in range(0, EC, 2):
                ps = psum.tile([P, 2, M], f32, tag="sc")
                for wj in range(2):
                    for dc in range(EC):
                        nc.tensor.matmul(
                            ps[:, wj, :],
                            lhsT=wq_sb[:, dc, (w0 + wj) * P : (w0 + wj + 1) * P],
                            rhs=xq_sb[:, dc, :],
                            start=(dc == 0),
                            stop=(dc == EC - 1),
                        )
                nc.vector.tensor_copy(qT_sb[:, w0 : w0 + 2, :], ps[:])

            # (load W_O early; DMA overlaps the attention loop)
            wo_sb = wpool.tile([P, EC, D], bf16, tag="w")
            nc.sync.dma_start(wo_sb[:], wo.rearrange("p (o e) -> p o e", o=EC))

            # ---- Phase D: attention, chunks j-major (gather completion
            # order) ----
            kT_views = [
                t.rearrange("(g p) (o k) -> g p o k", g=G, k=P) for t in kT_out
            ]
            v_views = [t.rearrange("(g p) e -> g p e", p=P) for t in v_out]

            sched = [(g, j) for j in range(G) for g in range(G)]
            NCHUNK = len(sched)

            # resident ctx head-pairs (accumulate in PSUM across all chunks)
            N_RES = 1
            cres = [
                psum.tile([P, M], f32, tag=f"cres{hp}", bufs=1, name=f"cres{hp}")
                for hp in range(N_RES)
            ]

            def emit_ctx_pair(vt, et, ci, hp):
                # one head-pair's context matmuls (col-packed, concurrent)
                if hp < N_RES:
                    cps = cres[hp]
                    st, sp = (ci == 0), (ci == NCHUNK - 1)
                else:
                    cps = psum.tile([P, M], f32, tag="cpsx", bufs=2, name="cpsx")
                    st, sp = True, True
                nc.tensor.matmul(
                    cps[0:DK, :],
                    lhsT=vt[:, (2 * hp) * DK : (2 * hp + 1) * DK],
                    rhs=et[:, 2 * hp, :],
                    start=st,
                    stop=sp,
                    tile_position=(0, 0),
                    skip_group_check=True,
                )
                nc.tensor.matmul(
                    cps[DK:P, :],
                    lhsT=vt[:, (2 * hp + 1) * DK : (2 * hp + 2) * DK],
                    rhs=et[:, 2 * hp + 1, :],
                    start=st,
                    stop=sp,
                    tile_position=(0, 64),
                    skip_group_check=True,
                )
                if hp >= N_RES:
                    if ci == 0:
                        nc.vector.tensor_copy(ctx_sb[:, hp, :], cps[:])
                    else:
                        nc.vector.tensor_tensor(
                            ctx_sb[:, hp, :], ctx_sb[:, hp, :], cps[:], OP.add
                        )

            def finalize_a(prev):
                # reciprocal of the previous chunk's denominator (DVE)
                _, _, _, dps, rf, _ = prev
                nc.vector.reciprocal_approx_fast(rf[:], dps[:])

            def finalize_b(prev):
                # cast recip to bf16 (ScalarE) + normalize: DVE heads 0-11
                # (one op), GpSimd heads 12-15
                _, et, _, _, rf, rb = prev
                nc.scalar.copy(rb[:], rf[:])
                nc.vector.tensor_tensor(
                    et[:, :12, :],
                    et[:, :12, :],
                    rb[:, None, :].to_broadcast((P, 12, M)),
                    OP.mult,
                )
                nc.gpsimd.tensor_tensor(
                    et[:, 12:, :],
                    et[:, 12:, :],
                    rb[:, None, :].to_broadcast((P, H - 12, M)),
                    OP.mult,
                )

            prev = None
            for ci, (g, j) in enumerate(sched):
                kt = kvstr.tile([P, EC, P], bf16, tag="kt")
                nc.sync.dma_start(kt[:], kT_views[j][g])
                vt = kvstr.tile([P, D], bf16, tag="vt")
                nc.sync.dma_start(vt[:], v_views[j][g])

                et = expp.tile([P, H, M], bf16, tag="exp")
                dps = psum.tile([P, M], f32, tag="dps", bufs=1)
                # score waves: 2 heads row-packed (PE row-tiles (0,0)/(64,0)
                # run concurrently); exp on ScalarE per wave; D identity
                # matmuls for wave w-1 slot in behind wave w's scores. The
                # PREVIOUS chunk's finalize (recip/cast/normalize) rides in
                # waves 0-1 and its ctx pairs fill waves 4-7, so the exp
                # train never stalls at a chunk boundary.
                for w in range(H // 2):
                    ps = psum.tile([P, 2, M], f32, tag="sc")
                    nc.tensor.matmul(
                        ps[:, 0, :],
                        lhsT=kt[0:DK, w, :],
                        rhs=qT_sb[0:DK, w, :],
                        start=True,
                        stop=True,
                    )
                    nc.tensor.matmul(
                        ps[:, 1, :],
                        lhsT=kt[DK:P, w, :],
                        rhs=qT_sb[DK:P, w, :],
                        start=True,
                        stop=True,
                    )
                    nc.scalar.activation(
                        et[:, 2 * w : 2 * w + 2, :], ps[:], AF.Exp, scale=0.125
                    )
                    if w == 0 and prev is not None:
                        finalize_a(prev)
                    if w == 1 and prev is not None:
                        finalize_b(prev)
                    if w >= 1:
                        for hh in (2 * (w - 1), 2 * w - 1):
                            nc.tensor.matmul(
                                dps[:],
                                lhsT=id_bf[:],
                                rhs=et[:, hh, :],
                                start=(hh == 0),
                                stop=False,
                                skip_group_check=True,
                            )
                    if w >= 4 and prev is not None:
                        emit_ctx_pair(prev[0], prev[1], prev[2], 2 * (w - 4))
                        emit_ctx_pair(prev[0], prev[1], prev[2], 2 * (w - 4) + 1)

                for hh in (H - 2, H - 1):
                    nc.tensor.matmul(
                        dps[:],
                        lhsT=id_bf[:],
                        rhs=et[:, hh, :],
                        start=False,
                        stop=(hh == H - 1),
                        skip_group_check=True,
                    )
                rf = smal.tile([P, M], f32, tag="rf", bufs=2)
                rb = smal.tile([P, M], bf16, tag="rb", bufs=2)
                prev = (vt, et, ci, dps, rf, rb)

            finalize_a(prev)
            finalize_b(prev)
            for hp in range(HP):
                emit_ctx_pair(prev[0], prev[1], prev[2], hp)
            # evacuate resident pairs
            for hp in range(N_RES):
                nc.vector.tensor_copy(ctx_sb[:, hp, :], cres[hp][:])

            # ---- Phase E: output projection + residual + LayerNorm ----
            ctx_bf = const.tile([P, HP, M], bf16)
            nc.vector.tensor_copy(ctx_bf[:], ctx_sb[:])
            res_view = xres.rearrange("(o p) e -> o p e", p=P)
            out_view = out.rearrange("(o p) e -> o p e", p=P)
            for qc in range(M // P):
                rest = resp.tile([P, D], f32, tag="res")
                nc.sync.dma_start(rest[:], res_view[qc])
                xsb = lnp.tile([P, D], f32, tag="x")
                ps = psum.tile([P, 2, M], f32, tag="sc")
                for eh in range(2):
                    for vc in range(EC):
                        nc.tensor.matmul(
                            ps[:, eh, :],
                            lhsT=ctx_bf[:, vc, qc * P : (qc + 1) * P],
                            rhs=wo_sb[:, vc, eh * M : (eh + 1) * M],
                            start=(vc == 0),
                            stop=(vc == EC - 1),
                        )
                for eh in range(2):
                    nc.vector.tensor_tensor(
                        xsb[:, eh * M : (eh + 1) * M],
                        ps[:, eh, :],
                        rest[:, eh * M : (eh + 1) * M],
                        OP.add,
                    )

                # LN stats on ScalarE: copy-with-accum -> sum(x); center via
                # per-partition bias; square-with-accum -> sum((x-mu)^2).
                # The activation data outputs are throwaways dumped into
                # tiles that are dead (or about to be overwritten).
                xc = lnp.tile([P, D], f32, tag="xc", bufs=1)
                mu_r = smal.tile([P, 1], f32, tag="mu")
                nc.scalar.activation(xc[:], xsb[:], AF.Identity, accum_out=mu_r[:])
                mu_neg = smal.tile([P, 1], f32, tag="mu2")
                nc.vector.tensor_scalar_mul(mu_neg[:], mu_r[:], -1.0 / D)
                nc.scalar.activation(xc[:], xsb[:], AF.Identity, bias=mu_neg[:])
                var_r = smal.tile([P, 1], f32, tag="var")
                nc.scalar.activation(xsb[:], xc[:], AF.Square, accum_out=var_r[:])
                veps = smal.tile([P, 1], f32, tag="veps")
                nc.vector.tensor_scalar(
                    veps[:], var_r[:], 1.0 / D, LN_EPS, OP.mult, OP.add
                )
                iv2 = smal.tile([P, 1], f32, tag="iv2")
                nc.vector.reciprocal_approx_fast(iv2[:], veps[:])
                inv = smal.tile([P, 1], f32, tag="inv")
                nc.scalar.activation(inv[:], iv2[:], AF.Sqrt)
                nc.vector.scalar_tensor_tensor(
                    xc[:], xc[:], inv[:], gam_sb[:], OP.mult, OP.mult
                )
                nc.vector.tensor_tensor(xsb[:], xc[:], bet_sb[:], OP.add)
                nc.sync.dma_start(out_view[qc], xsb[:])

    nc.compile()
    return nc


def _get_nc():
    if "nc" not in _CACHE:
        _CACHE["nc"] = _build()
    return _CACHE["nc"]


def _pmaj(a2d):
    """[D_out, N] -> partition-major [P, EC*N] so device DMA reads are
    128 fat contiguous rows: row p holds [o, n] for o = D_out chunk."""
    d, n = a2d.shape
    return np.ascontiguousarray(
        a2d.reshape(d // P, P, n).transpose(1, 0, 2).reshape(P, (d // P) * n)
    )


def _in_maps(input_Q, input_K, input_V, W_Q, W_K, W_V, W_O, ln_gamma, ln_beta):
    bf = ml_dtypes.bfloat16
    f32 = np.float32
    Q_ = np.asarray(input_Q, dtype=f32)
    K_ = np.asarray(input_K, dtype=f32)
    V_ = np.asarray(input_V, dtype=f32)
    wq_b = _pmaj(np.asarray(W_Q, dtype=f32).astype(bf))
    wk_b = _pmaj(np.asarray(W_K, dtype=f32).astype(bf))
    wv_b = _pmaj(np.asarray(W_V, dtype=f32).astype(bf))
    wo_b = _pmaj(np.asarray(W_O, dtype=f32).astype(bf))
    gam_b = np.ascontiguousarray(
        np.broadcast_to(np.asarray(ln_gamma, dtype=f32), (P, D))
    )
    bet_b = np.ascontiguousarray(
        np.broadcast_to(np.asarray(ln_beta, dtype=f32), (P, D))
    )
    maps = []
    for c in range(8):
        b, r = divmod(c, G)
        sl = slice(r * M, (r + 1) * M)
        maps.append(
            {
                "xqT": _pmaj(np.ascontiguousarray(Q_[b, sl].T).astype(bf)),
                "xkT": _pmaj(np.ascontiguousarray(K_[b, sl].T).astype(bf)),
                "xvT": _pmaj(np.ascontiguousarray(V_[b, sl].T).astype(bf)),
                "xres": np.ascontiguousarray(Q_[b, sl]),
                "wq": wq_b,
                "wk": wk_b,
                "wv": wv_b,
                "wo": wo_b,
                "gam": gam_b,
                "bet": bet_b,
            }
        )
    return maps


def _assemble(results):
    B = 2
    out = np.empty((B, SK, D), np.float32)
    for c in range(8):
        b, r = divmod(c, G)
        out[b, r * M : (r + 1) * M] = results[c]["out"]
    return out


def run_traced(trace=False, **inputs):
    """Run on HW; returns (output, BassKernelResults)."""
    from concourse.bass_utils import run_bass_kernel_spmd

    nc = _get_nc()
    maps = _in_maps(**inputs)
    res = run_bass_kernel_spmd(nc, maps, list(range(8)), trace=trace)
    return _assemble(res.results), res


def kernel(**inputs) -> np.ndarray:
    out, _ = run_traced(trace=False, **inputs)
    return out
